# revision 1
# baseline (speedup 1.0000x reference)
"""Trainium2 Bass kernel for nn_SampleRepresentativeCalculator.

Shards the Z (band) axis of all [Z,Y,X] tensors across 8 NeuronCores
(28 bands per core), runs a fused elementwise pipeline per core, and
gathers the full outputs. Returns (reps, bin_centers) like the reference.

Math notes (all fp32, validated bit-level against the reference):
  step = 2*m+1 in {1,3,5,7,9}; the host sends c = fl(1/step) (a lossless
  re-encoding of the int m in {0..4}).  On device:
    k    = rint(r*c)            (magic-number round; bit-matches
                                 jnp.round(r/step) for these divisors)
    step = rint(recip_approx(c)); qres = k*step
    bc   = p + qres, overwritten with o where m==0 (copy_predicated)
    d    = p - bc;  adj = where(|d|<=th, phi/(th+eps)*d,
                                 psi*sign(d)*(|d|-th)/(|d|+eps))
    reps = bc + adj
"""
import numpy as np

import concourse.bass as bass
import concourse.tile as tile
from concourse import bacc, mybir
from concourse.bass_utils import run_bass_kernel_spmd
from concourse.dve_ops import (
    DveOp, OPS, CUSTOM_DVE_SPECS, _SUB_OPCODE_FOR_NAME, _CUSTOM_DVE_ROW_BASE,
    has_src1,
)
from concourse.dve_spec import (
    Spec, Src0, Src1, C0, C1, C2, Zero, lower, maxx, minn, eq, Bin, AluOp,
)
from concourse.dve_uop import DveOpSpec

F32 = np.float32
MAGIC = 12582912.0          # 1.5 * 2**23 : rint(x) == (x + M) - M for |x|<2^22
RC0 = -0.23549792           # reciprocal-approx Chebyshev seed constants
RC1 = 2.0017324

Z, Y, X = 224, 256, 512
N_CORES = 8
ZPC = Z // N_CORES          # 28 bands per core
FD = 1024                   # free dim per tile; one tile == half a band... see below
ROWS = ZPC * Y * X // FD    # 3584 rows of FD per core
N_TILES = ROWS // 128       # 28 tiles of [128, FD]; tile t == band t of the core


def _register(name, spec, subdim=False):
    """Runtime-register a custom DVE op (mirrors DveOp.compile sha pinning)."""
    if name in _SUB_OPCODE_FOR_NAME:
        for op in OPS:
            if op.name == name:
                return op
        raise RuntimeError(name)
    opcode = _CUSTOM_DVE_ROW_BASE + len(OPS)
    assert opcode < 0x20, "custom DVE row overflow"
    shas = {}
    for ver in ("v3", "v4"):
        s = DveOpSpec(name=name, opcode=opcode, uops=lower(spec, ver=ver),
                      rd1_en=has_src1(spec))
        shas[ver] = s.sha(ver)
    op = DveOp(name, spec, subdim=subdim, uops_sha=shas)
    OPS.append(op)
    CUSTOM_DVE_SPECS[name] = spec
    _SUB_OPCODE_FOR_NAME[name] = opcode
    return op


def _bitnot_f32(x):
    x = np.ascontiguousarray(x, F32)
    return (~x.view(np.int32)).view(F32)


def _ref_krint(in0, in1, c0, c1, c2):
    q = (in0 * in1).astype(F32)
    return ((q + F32(c0)).astype(F32) - F32(c0)).astype(F32)


def _ref_qres(in0, in1, c0, c1, c2):
    nx = _bitnot_f32(in0)
    y0 = (nx * F32(c0)).astype(F32)
    y1 = (y0 * (F32(c1) - (in0 * y0).astype(F32)).astype(F32)).astype(F32)
    s = ((y1 + F32(c2)).astype(F32) - F32(c2)).astype(F32)
    return (s * in1).astype(F32)


def _ref_adj(in0, in1, c0, c1, c2):
    x2 = np.minimum(np.maximum(in0, F32(c1)), F32(c2))
    num = (in0 - x2).astype(F32)
    b2 = (num * in1).astype(F32)
    c01 = (num == 0).astype(F32)
    b1m = ((in0 * c0).astype(F32) * c01).astype(F32)
    return (b1m + b2).astype(F32)


SRC_KRINT = _register(
    "SRC_KRINT_ANT", Spec(body=(Src0 * Src1 + C0) - C0, reference=_ref_krint))

_nx = Bin(AluOp.BITWISE_NOT, Src0, Src0)
_y0 = _nx * C0
_y1 = _y0 * (C1 - Src0 * _y0)
QRES_STEP = _register(
    "QRES_STEP_ANT",
    Spec(body=((_y1 + C2) - C2) * Src1, reference=_ref_qres))

_num = Src0 - minn(maxx(Src0, C1), C2)
ADJ_OP = _register(
    "ADJ_SRC_ANT",
    Spec(body=(Src0 * C0) * eq(_num, Zero) + _num * Src1, reference=_ref_adj))


def build_kernel(th, bufs=2):
    nc = bacc.Bacc(
        "TRN2",
        target_bir_lowering=False,
        debug=False,
        enable_asserts=False,
        num_devices=N_CORES,
    )
    f32 = mybir.dt.float32
    o_d = nc.dram_tensor("o", [ROWS, FD], f32, kind="ExternalInput")
    p_d = nc.dram_tensor("p", [ROWS, FD], f32, kind="ExternalInput")
    c_d = nc.dram_tensor("c", [ROWS, FD], f32, kind="ExternalInput")
    ps_d = nc.dram_tensor("ps", [128, 2 * N_TILES], f32, kind="ExternalInput")
    bc_d = nc.dram_tensor("bc", [ROWS, FD], f32, kind="ExternalOutput")
    reps_d = nc.dram_tensor("reps", [ROWS, FD], f32, kind="ExternalOutput")

    o_t = o_d[:].rearrange("(t p) f -> t p f", p=128)
    p_t = p_d[:].rearrange("(t p) f -> t p f", p=128)
    c_t = c_d[:].rearrange("(t p) f -> t p f", p=128)
    bc_t = bc_d[:].rearrange("(t p) f -> t p f", p=128)
    reps_t = reps_d[:].rearrange("(t p) f -> t p f", p=128)

    with tile.TileContext(nc) as tc:
        with tc.tile_pool(name="consts", bufs=1) as cpool, \
             tc.tile_pool(name="io", bufs=bufs) as iop, \
             tc.tile_pool(name="tmp", bufs=bufs) as tp:
            ps = cpool.tile([128, 2 * N_TILES], f32)
            nc.sync.dma_start(ps[:], ps_d[:])
            neg2 = cpool.tile([128, 1], f32)
            nc.gpsimd.memset(neg2[:], -2.0)
            epsb = cpool.tile([128, 1], f32)
            nc.gpsimd.memset(epsb[:], 1e-8)

            for t in range(N_TILES):
                ot = iop.tile([128, FD], f32, tag="o")
                pt = iop.tile([128, FD], f32, tag="p")
                ct = iop.tile([128, FD], f32, tag="c")
                nc.sync.dma_start(ot[:], o_t[t])
                nc.sync.dma_start(pt[:], p_t[t])
                nc.sync.dma_start(ct[:], c_t[t])

                phi_ap = ps[:, t:t + 1]
                psi_ap = ps[:, N_TILES + t:N_TILES + t + 1]

                r = tp.tile([128, FD], f32, tag="r")
                nc.vector.tensor_sub(r[:], ot[:], pt[:])

                # mask: relu(4c-2) = 2.0 iff c==1 (m==0), else 0 -> uint8
                m0 = tp.tile([128, FD], mybir.dt.uint8, tag="m0")
                nc.scalar.activation(m0[:], ct[:],
                                     mybir.ActivationFunctionType.Relu,
                                     bias=neg2[:], scale=4.0)

                k = tp.tile([128, FD], f32, tag="k")
                nc.vector._custom_dve(SRC_KRINT, out=k[:], in0=r[:],
                                      in1=ct[:], s0=MAGIC)

                qres = tp.tile([128, FD], f32, tag="qres")
                nc.vector._custom_dve(QRES_STEP, out=qres[:], in0=ct[:],
                                      in1=k[:], s0=RC0, s1=RC1, imm2=MAGIC)

                bct = tp.tile([128, FD], f32, tag="bc")
                nc.vector.tensor_add(bct[:], pt[:], qres[:])
                nc.vector.copy_predicated(bct[:], m0[:], ot[:])
                nc.sync.dma_start(bc_t[t], bct[:])

                d = tp.tile([128, FD], f32, tag="d")
                nc.gpsimd.tensor_sub(d[:], pt[:], bct[:])

                ad = tp.tile([128, FD], f32, tag="ad")
                nc.scalar.activation(ad[:], d[:],
                                     mybir.ActivationFunctionType.Abs)
                den = tp.tile([128, FD], f32, tag="den")
                nc.scalar.activation(den[:], ad[:],
                                     mybir.ActivationFunctionType.Identity,
                                     bias=epsb[:])

                rf = tp.tile([128, FD], f32, tag="rf")
                nc.vector.reciprocal_approx_fast(rf[:], den[:])

                rfp = tp.tile([128, FD], f32, tag="rfp")
                nc.scalar.activation(rfp[:], rf[:],
                                     mybir.ActivationFunctionType.Copy,
                                     bias=0.0, scale=psi_ap)

                adj = tp.tile([128, FD], f32, tag="adj")
                nc.vector._custom_dve(ADJ_OP, out=adj[:], in0=d[:],
                                      in1=rfp[:], s0=phi_ap, s1=-th, imm2=th)

                rep = tp.tile([128, FD], f32, tag="rep")
                nc.gpsimd.tensor_add(rep[:], bct[:], adj[:])
                nc.sync.dma_start(reps_t[t], rep[:])
    nc.compile()
    return nc


_NC_CACHE = {}


def _get_nc(th):
    key = float(th)
    if key not in _NC_CACHE:
        _NC_CACHE[key] = build_kernel(key)
    return _NC_CACHE[key]


def kernel(original_samples, predicted_samples, max_errors, phi, psi, theta,
           _run_kwargs=None, _return_raw=False):
    o = np.ascontiguousarray(original_samples, F32)
    p = np.ascontiguousarray(predicted_samples, F32)
    mi = np.ascontiguousarray(max_errors, np.int32)
    phi = np.asarray(phi, F32)
    psi = np.asarray(psi, F32)
    th = float(np.asarray(theta, F32).reshape(-1)[0])

    # host-side lossless re-encode of m -> c = fl(1/(2m+1))
    c = (F32(1.0) / (2 * mi + 1).astype(F32)).astype(F32)

    the = (F32(th) + F32(1e-8)).astype(F32)
    phi_c = (phi / the).astype(F32)
    in_maps = []
    for i in range(N_CORES):
        z0 = i * ZPC
        ps = np.empty((128, 2 * N_TILES), F32)
        ps[:, :N_TILES] = phi_c[z0:z0 + ZPC][None, :]
        ps[:, N_TILES:] = psi[z0:z0 + ZPC][None, :]
        in_maps.append(dict(
            o=o[z0:z0 + ZPC].reshape(ROWS, FD),
            p=p[z0:z0 + ZPC].reshape(ROWS, FD),
            c=c[z0:z0 + ZPC].reshape(ROWS, FD),
            ps=ps,
        ))

    nc = _get_nc(th)
    res = run_bass_kernel_spmd(nc, in_maps, list(range(N_CORES)),
                               **(_run_kwargs or {}))

    reps = np.empty((Z, Y, X), F32)
    bc = np.empty((Z, Y, X), F32)
    for i in range(N_CORES):
        z0 = i * ZPC
        reps[z0:z0 + ZPC] = res.results[i]["reps"].reshape(ZPC, Y, X)
        bc[z0:z0 + ZPC] = res.results[i]["bc"].reshape(ZPC, Y, X)
    if _return_raw:
        return (reps, bc), res
    return reps, bc
